# revision 68
# baseline (speedup 1.0000x reference)
"""Trainium2 Bass kernel: GSpade node embedding (fp8 DoubleRow scan).

Computation (see reference):
  - bidirectional tanh-RNN (hidden 512/dir) over T=32768 tokens grouped into
    N=2048 contiguous ragged segments (sorted group ids in `masks`)
  - mean-pool hidden states per segment -> pooled [N, 1024]
  - out = [x @ Wx.T + bx | pooled]  -> [N, 2048]

Sharding (8 NeuronCores, SPMD single program):
  cores 0-3 forward RNN, 4-7 backward.  Segments sorted by length desc and
  striped by rank (stripe c = core % 4 owns ranks 4i+c -> 512 lanes/core).
  Lanes end-aligned to the shared schedule L_i = len at rank 4i (zero-prefix
  padding keeps h == 0 until the first real token).  Active lanes form a
  prefix; nt[t] shrinks with t.

Per-core scan, all fp8e4m3 with DoubleRow matmuls (2 contraction slots per
partition, 0.5 cycles/row):
  - input proj + bias: tokens packed [65, 2, S]; slots (p<64, i) carry the
    128 token features, slot (64, 0) carries the real-token mask so the bias
    row of the packed W_ih folds the masked bias into the same matmuls.
    4 DR matmuls per part.
  - recurrence: W_hh.T packed [128, 4, 512]; 8 DR matmuls per part
    (4 j-chunks x 2 k-supers) reading the fp8 h of the previous step.
  - tanh: one ACT instruction per part, PSUM -> fp8 h (double-buffered).
  - mean-pool acc: chunks 0-1 via DR identity matmuls into a 2-bank PSUM
    accumulator; chunks 2-3 via DVE adds into SBUF fp32.
Steps with n >= 128 lanes are split into two lane-halves (A/B): ACT runs
half X's tanh while PE runs the other half's matmuls, hiding the serial
recurrence under the ACT tanh floor.  acc work is emitted two parts late so
PE/DVE never queue behind an in-flight ACT.  The bf16 x-projection (fp8
would exceed tolerance) is injected one matmul per part into PE slack,
using the 2 PSUM banks the accumulator does not occupy.
"""

import ml_dtypes
import numpy as np

import concourse.bacc as bacc
import concourse.mybir as mybir
from concourse.tile import TileContext
from concourse.bass_utils import run_bass_kernel_spmd

FP32 = mybir.dt.float32
BF16 = mybir.dt.bfloat16
F8 = mybir.dt.float8e4
DR = mybir.MatmulPerfMode.DoubleRow
Tanh = mybir.ActivationFunctionType.Tanh
E4NP = ml_dtypes.float8_e4m3
BFNP = ml_dtypes.bfloat16

N_GROUPS = 2048
D_SEQ = 128
H = 512           # hidden per direction
HC = 4            # hidden chunks of 128
D_PROJ = 1024
N_CORES = 8
LANES = 512       # segments per scan core
XROWS = N_GROUPS // N_CORES  # x-projection rows per core
SPLIT_MIN = 128   # lane-split steps with n >= this

_program_cache: dict = {}


def _dma_chunks(nt, targets=(768, 1536, 2048)):
    """Group steps into DMA chunks; early chunks smaller so step 0 starts fast."""
    chunks = []
    t0 = 0
    cols = 0
    for t, n in enumerate(nt):
        target = targets[min(len(chunks), len(targets) - 1)]
        if cols > 0 and cols + n > target:
            chunks.append((t0, t))
            t0, cols = t, 0
        cols += n
    chunks.append((t0, len(nt)))
    return chunks


def _parts_of(nt):
    """Flat list of (t, lane_off, nX) parts; big steps split in half."""
    parts = []
    for t, n in enumerate(nt):
        if n >= SPLIT_MIN:
            na = (n + 1) // 2
            parts.append((t, 0, na))
            parts.append((t, na, n - na))
        else:
            parts.append((t, 0, n))
    return parts


def _build_program(nt):
    nt = list(nt)
    steps = len(nt)
    off = np.concatenate([[0], np.cumsum(nt)]).astype(int)
    S = int(off[-1])
    parts = _parts_of(nt)

    nc = bacc.Bacc("TRN2", target_bir_lowering=False, debug=False,
                   num_devices=N_CORES)

    # token stream [65, 2, 512+S(pad16)]: cols 0:512 of each i-half hold the
    # packed W_ih (bias row folded), so one DMA pair covers weights + step 0.
    # The i-half stride must be 16-aligned (DoubleRow LDWEIGHTS ISA rule).
    SW = H + ((S + 15) // 16) * 16
    xtok_d = nc.dram_tensor("xtok", [65, 2 * SW], F8, kind="ExternalInput")
    whh_d = nc.dram_tensor("whh", [128, HC * H], F8, kind="ExternalInput")
    ida_d = nc.dram_tensor("ida", [128, 2 * 128], F8, kind="ExternalInput")
    idb_d = nc.dram_tensor("idb", [128, 2 * 128], F8, kind="ExternalInput")
    # 1/len per lane, duplicated x2 so a single DVE mul covers a chunk pair
    invl_d = nc.dram_tensor("invl", [128, 2 * LANES], FP32, kind="ExternalInput")
    xT_d = nc.dram_tensor("xT", [128, 4 * XROWS], BF16, kind="ExternalInput")
    wxT_d = nc.dram_tensor("wxT", [128, 4 * D_PROJ], BF16, kind="ExternalInput")
    bxrow_d = nc.dram_tensor("bxrow", [1, D_PROJ], BF16, kind="ExternalInput")
    ones_d = nc.dram_tensor("ones", [1, 128], BF16, kind="ExternalInput")

    xp_d = nc.dram_tensor("xp", [XROWS, D_PROJ], BF16, kind="ExternalOutput")
    # lane-major pooled: [p, lane*4 + c] -> feature c*128+p of lane
    pooled2_d = nc.dram_tensor("pooled2", [128, 4 * LANES], BF16, kind="ExternalOutput")

    with TileContext(nc) as tc:
        with (
            tc.tile_pool(name="sb", bufs=1) as sb,
            tc.tile_pool(name="ps", bufs=2, space="PSUM") as psp,
            tc.tile_pool(name="psx", bufs=1, space="PSUM") as psxp,
            tc.tile_pool(name="xps", bufs=1, space="PSUM") as xpsp,
            tc.tile_pool(name="accps", bufs=1, space="PSUM") as accp,
        ):
            # tanh table warm-up: runs under the DMA shadow so the table set
            # is resident before step 0.
            warm_sb = sb.tile([128, 8], FP32, tag="warm", name="warm")
            nc.vector.memset(warm_sb[:, :], 0.0)
            nc.scalar.activation(warm_sb[:, :], warm_sb[:, :], Tanh)

            # ---- persistent SBUF tiles + loads (step-0-critical first) ----
            chunks = _dma_chunks(nt)
            xtok_sb = sb.tile([65, 2 * SW], F8, tag="xtok", name="xtok")

            xtok3_sb = xtok_sb.rearrange("p (i s) -> p i s", i=2)
            xtok3_d = xtok_d.rearrange("p (i s) -> p i s", i=2)

            def xtok_range(a, b):
                # one 3D DMA moves both i-halves (one HWDGE slot, not two)
                nc.sync.dma_start(out=xtok3_sb[:, :, a:b],
                                  in_=xtok3_d[:, :, a:b])

            def xtok_chunk(ci):
                t0, t1 = chunks[ci]
                a = 0 if ci == 0 else int(off[t0]) + H
                xtok_range(a, int(off[t1]) + H)

            xtok_chunk(0)
            whh_sb = sb.tile([128, HC * H], F8, tag="whh", name="whh")
            nc.sync.dma_start(out=whh_sb[:, :], in_=whh_d[:, :])
            if len(chunks) > 1:
                xtok_chunk(1)
            ida_sb = sb.tile([128, 2 * 128], F8, tag="ida", name="ida")
            idb_sb = sb.tile([128, 2 * 128], F8, tag="idb", name="idb")
            nc.sync.dma_start(out=ida_sb[:, :], in_=ida_d[:, :])
            nc.sync.dma_start(out=idb_sb[:, :], in_=idb_d[:, :])
            invl_sb = sb.tile([128, 2 * LANES], FP32, tag="invl", name="invl")
            nc.sync.dma_start(out=invl_sb[:, :], in_=invl_d[:, :])
            for ci in range(2, len(chunks)):
                xtok_chunk(ci)

            ones_sb = sb.tile([1, 128], BF16, tag="ones", name="ones")
            bx_sb = sb.tile([1, D_PROJ], BF16, tag="bx", name="bx")
            nc.sync.dma_start(out=ones_sb[:, :], in_=ones_d[:, :])
            nc.sync.dma_start(out=bx_sb[:, :], in_=bxrow_d[:, :])
            xT_sb = sb.tile([128, 4 * XROWS], BF16, tag="xT", name="xT")
            wx_sb = sb.tile([128, 4 * D_PROJ], BF16, tag="wx", name="wx")
            nc.sync.dma_start(out=xT_sb[:, :], in_=xT_d[:, :])
            nc.sync.dma_start(out=wx_sb[:, :], in_=wxT_d[:, :])

            # scan state (h triple-buffered: ACT(t) writes buf t%3 while
            # lagged acc still reads buf (t-1)%3 without WAR stalls)
            h_sb = [sb.tile([128, HC * LANES], F8, tag=f"h{p}", name=f"h{p}")
                    for p in range(3)]
            acc2_sb = sb.tile([128, 2 * LANES], FP32, tag="acc2", name="acc2")
            xp_sb = [sb.tile([128, D_PROJ], BF16, tag=f"xp{b}", name=f"xpsb{b}")
                     for b in range(2)]
            po_sb = sb.tile([128, 4 * LANES], BF16, tag="po", name="po")  # [p, c, n]

            whh3 = whh_sb.rearrange("p (k m) -> p k m", k=HC)
            ida3 = ida_sb.rearrange("p (i m) -> p i m", i=2)
            idb3 = idb_sb.rearrange("p (i m) -> p i m", i=2)
            xtok3 = xtok3_sb
            acc2v = acc2_sb.rearrange("p (c n) -> p c n", c=2)
            h3 = [h.rearrange("p (c n) -> p c n", c=HC) for h in h_sb]
            pov = po_sb.rearrange("p (c n) -> p c n", c=4)
            pov_d = pooled2_d.rearrange("p (c n) -> p c n", c=4)
            invl2 = invl_sb.rearrange("p (c n) -> p c n", c=2)

            acc_ps = accp.tile([128, 2 * LANES], FP32, tag="acc", name="acc")
            accv = acc_ps.rearrange("p (c n) -> p c n", c=2)

            # ---- x-projection micro-op queue (injected into PE slack) ----
            # 4 groups of (bias + 4 kc) bf16 matmuls; each group -> one DVE
            # copy to bf16 SBUF; DMA per bc block.
            xproj_ops = []

            def _xproj_group(bc, jh):
                ps = xpsp.tile([128, H], FP32, tag="xps", name="xps")
                ops = [lambda ps=ps, bc=bc, jh=jh: nc.tensor.matmul(
                    ps[:, :], ones_sb[0:1, :], bx_sb[0:1, jh * H:(jh + 1) * H],
                    start=True, stop=False)]
                for kc in range(4):
                    ops.append(lambda ps=ps, bc=bc, jh=jh, kc=kc: nc.tensor.matmul(
                        ps[:, :],
                        xT_sb[:, kc * XROWS + bc * 128:kc * XROWS + (bc + 1) * 128],
                        wx_sb[:, kc * D_PROJ + jh * H:kc * D_PROJ + (jh + 1) * H],
                        start=False, stop=(kc == 3)))
                ops.append(lambda ps=ps, bc=bc, jh=jh: nc.vector.tensor_copy(
                    xp_sb[bc][:, jh * H:(jh + 1) * H], ps[:, :]))
                if jh == 1:
                    ops.append(lambda bc=bc: nc.sync.dma_start(
                        out=xp_d[bc * 128:(bc + 1) * 128, :], in_=xp_sb[bc][:, :]))
                return ops

            for bc in range(2):
                for jh in range(2):
                    xproj_ops.extend(_xproj_group(bc, jh))
            xproj_ops.reverse()  # pop() from the front

            # ---- scan: express-lane schedule ----
            # The XL longest lanes (the only ones alive past step TX) run as
            # their own one-instruction-per-step chain ("X") from step 0,
            # interleaved into the bulk ("Y") ACT stream at >= ring-latency
            # spacing, so the serial tail hides under the bulk's tanh
            # throughput instead of running after it.
            import heapq
            TX = next((t for t, n in enumerate(nt) if n < SPLIT_MIN), steps)
            XL = nt[TX] if TX < steps else 0

            seq = []          # (t, o0, nX, kind)
            est_act = 0.0
            x_last = -1e9
            xi = 0
            X_SPACING = 520.0

            def est_cost(n):
                return 3.33 * n + 235.0

            def push_x():
                nonlocal est_act, x_last, xi
                nXp = min(nt[xi], XL)
                seq.append((xi, 0, nXp, "X"))
                x_last = est_act
                est_act += est_cost(nXp)
                xi += 1

            if XL > 0:
                push_x()
            for t in range(TX if XL > 0 else steps):
                nY = nt[t] - XL
                if nY >= SPLIT_MIN:
                    halves = [(XL, (nY + 1) // 2),
                              (XL + (nY + 1) // 2, nY - (nY + 1) // 2)]
                else:
                    halves = [(XL, nY)] if nY > 0 else []
                for (o0, nXp) in halves:
                    if xi < steps and XL > 0 and est_act - x_last >= X_SPACING:
                        push_x()
                    seq.append((t, o0, nXp, "Y"))
                    est_act += est_cost(nXp)
            while XL > 0 and xi < steps:
                push_x()
                est_act = max(est_act, x_last + X_SPACING)

            nparts = len(seq)
            acc_queue = []  # heap of (emit_at_part_idx, seq#, thunk)
            qseq = [0]

            def qpush(at, thunk):
                heapq.heappush(acc_queue, (at, qseq[0], thunk))
                qseq[0] += 1

            def emit_fin_mul(b0, b1, pair):
                # pooled finalize for retired lanes [b0, b1), chunk pair.
                # The SBUF-side pair of the late (express) ranges runs on the
                # idle GPSIMD so the end-of-kernel DVE cascade shortens.
                src = accv[:, 0:2, b0:b1] if pair == 0 else acc2v[:, 0:2, b0:b1]
                nc.vector.tensor_mul(pov[:, 2 * pair:2 * pair + 2, b0:b1],
                                     src, invl2[:, :, b0:b1])

            def emit_fin_dma(b0, b1):
                nc.sync.dma_start(out=pov_d[:, :, b0:b1], in_=pov[:, :, b0:b1])

            def emit_fin(b0, b1):
                for pair in range(2):
                    emit_fin_mul(b0, b1, pair)
                emit_fin_dma(b0, b1)

            def emit_acc(k):
                t, o0, nX, _kind = seq[k]
                hv = h3[t % 3]
                first = (k == 0)
                last_a = (k == nparts - 1)
                # PE: chunks 0,1 -> PSUM acc via DR identity
                for c in range(2):
                    lhs = ida3 if c % 2 == 0 else idb3
                    nc.tensor.matmul(accv[:, c, o0:o0 + nX],
                                     lhs[:, :, :], hv[:, 0:2, o0:o0 + nX],
                                     start=first, stop=last_a,
                                     perf_mode=DR)
                # DVE: chunks 2,3 -> SBUF fp32
                if t == 0:
                    nc.vector.tensor_copy(acc2v[:, :, o0:o0 + nX],
                                          hv[:, 2:4, o0:o0 + nX])
                else:
                    nc.vector.tensor_add(acc2v[:, :, o0:o0 + nX],
                                         acc2v[:, :, o0:o0 + nX],
                                         hv[:, 2:4, o0:o0 + nX])

            FIN_BOUNDS = tuple(sorted({384, 256, 160, XL, 2}, reverse=True))
            fin_y = [LANES]   # descends to XL via Y retirements
            fin_x = [XL]      # descends to 0 via X retirements
            est_pe = [0.0]
            est_a2 = [0.0]
            last_xp = [-2]

            def emit_fins(nxt, k, bound_state, blo, bhi):
                for b in FIN_BOUNDS:
                    if blo <= b < bhi and nxt <= b < bound_state[0]:
                        lo, hi = b, bound_state[0]
                        for pair in range(2):
                            qpush(k + 3 + pair,
                                  lambda lo=lo, hi=hi, pair=pair:
                                  emit_fin_mul(lo, hi, pair))
                        qpush(k + 5, lambda lo=lo, hi=hi: emit_fin_dma(lo, hi))
                        bound_state[0] = b

            for k, (t, o0, nX, kind) in enumerate(seq):
                a = int(off[t])
                hw3 = h3[t % 3]
                hr3 = h3[(t + 2) % 3]
                if kind == "Y":
                    ps = psp.tile([128, HC * 256], FP32, tag="ps", name="ps")
                    ps3 = ps.rearrange("p (c n) -> p c n", c=HC)
                else:
                    ps = psxp.tile([128, HC * max(XL, 1)], FP32, tag="psx",
                                   name="psx")
                    ps3 = ps.rearrange("p (c n) -> p c n", c=HC)

                # psum-group flags follow zero regions (banks): Y tiles span 2
                # banks (chunks 0,1 | 2,3); the X tile fits in one bank.
                starts = (0, 2) if kind == "Y" else (0,)
                stops = (1, 3) if kind == "Y" else (3,)

                # input proj + bias (DR; lhsT is the W_ih block leading the
                # token stream)
                xv = xtok3[:, :, H + a + o0:H + a + o0 + nX]
                for jc in range(HC):
                    nc.tensor.matmul(ps3[:, jc, 0:nX],
                                     xtok3[:, :, jc * 128:(jc + 1) * 128], xv,
                                     start=(jc in starts),
                                     stop=(t == 0 and jc in stops),
                                     perf_mode=DR)
                # recurrence (DR)
                if t > 0:
                    for s in range(2):
                        hk = hr3[:, 2 * s:2 * s + 2, o0:o0 + nX]
                        for jc in range(HC):
                            nc.tensor.matmul(
                                ps3[:, jc, 0:nX],
                                whh3[:, 2 * s:2 * s + 2, jc * 128:(jc + 1) * 128],
                                hk,
                                start=False,
                                stop=(s == 1 and jc in stops),
                                perf_mode=DR)

                # lagged acc (2 parts back) keeps PE/DVE off in-flight ACT.
                # X parts stay lean (proj/rec/tanh only) so the express
                # chain's recurrence never queues behind bulk work on PE.
                est_pe[0] += 2.92 * nX
                if kind == "Y" or k >= nparts - 2:
                    while acc_queue and acc_queue[0][0] <= k:
                        heapq.heappop(acc_queue)[2]()
                    if (xproj_ops and est_a2[0] - est_pe[0] > 450.0
                            and k - last_xp[0] >= 2):
                        xproj_ops.pop()()
                        est_pe[0] += 213.0
                        last_xp[0] = k

                # tanh -> fp8 h
                nc.scalar.activation(hw3[:, 0:HC, o0:o0 + nX],
                                     ps3[:, 0:HC, 0:nX], Tanh)
                est_a2[0] += est_cost(nX)
                qpush(k + 2, lambda k=k: emit_acc(k))

                # progressive pooled finalize once a retirement boundary's acc
                # is emitted (lag 3); split muls keep DVE load smooth.
                if kind == "Y" and o0 + nX == nt[t]:
                    nxt = nt[t + 1] if t + 1 < steps else 0
                    emit_fins(max(nxt, XL), k, fin_y, XL, LANES)
                elif kind == "X":
                    nxt = min(nt[t + 1], XL) if t + 1 < steps else 0
                    emit_fins(nxt, k, fin_x, 0, XL)

            while acc_queue:
                heapq.heappop(acc_queue)[2]()
            while xproj_ops:
                xproj_ops.pop()()
            if fin_y[0] > XL:
                emit_fin(XL, fin_y[0])
            if fin_x[0] > 0:
                emit_fin(0, fin_x[0])

    nc.compile()
    return nc


def _get_program(nt):
    key = tuple(nt)
    if key not in _program_cache:
        _program_cache[key] = _build_program(nt)
    return _program_cache[key]


def _prepare(x, seqs, masks, W_ih_f, W_hh_f, b_f, W_ih_b, W_hh_b, b_b, Wx, bx):
    x = np.asarray(x, np.float32)
    seqs = np.asarray(seqs, np.float32)
    masks = np.asarray(masks).astype(np.int64)

    # ---- segment geometry (host) ----
    lens = np.bincount(masks, minlength=N_GROUPS).astype(np.int64)
    starts_all = np.concatenate([[0], np.cumsum(lens)[:-1]])
    order = np.argsort(-lens, kind="stable")
    sl = lens[order]
    L = sl[0::4].astype(np.int64)        # shared lane schedule (512)
    steps = int(L[0])
    nt = [int((L > t).sum()) for t in range(steps)]
    off = np.concatenate([[0], np.cumsum(nt)]).astype(int)
    S = int(off[-1])

    t_grid = np.arange(steps)[:, None]
    active = t_grid < L[None, :]         # [steps, LANES]

    seqs_pad = np.vstack([np.zeros((1, D_SEQ), np.float32), seqs])

    gid = [order[c::4] for c in range(4)]
    per_stripe = {}
    for c in range(4):
        lens_c = lens[gid[c]]
        starts_c = starts_all[gid[c]]
        pre = (L - lens_c)[None, :]      # zero-prefix length
        real = active & (t_grid >= pre)
        pos = t_grid - pre
        idx_f = np.where(real, starts_c[None, :] + pos, -1)
        idx_b = np.where(real, starts_c[None, :] + lens_c[None, :] - 1 - pos, -1)
        mask_flat = real[active].astype(np.float32)          # [S]
        feat_f = seqs_pad[idx_f[active] + 1].T               # [128, S]
        feat_b = seqs_pad[idx_b[active] + 1].T
        invl = np.ascontiguousarray(
            np.broadcast_to(
                np.tile((1.0 / lens_c).astype(np.float32), 2)[None, :],
                (128, 2 * LANES)))
        per_stripe[c] = (feat_f, feat_b, mask_flat, invl)

    SW = H + ((S + 15) // 16) * 16   # 16-aligned i-half stride (ISA rule)

    def pack_tokens(feat, mask_flat, W_ih, b):
        # [65, 2, SW]: leading 512 cols of each i-half = packed W_ih
        # ([p, i, m] = W_ih[m, 64i+p]; [64, 0, m] = b[m]); then the tokens
        # (slots (p<64, i) = feature 64i+p; (64,0) = mask; (64,1) = 0).
        xt = np.zeros((65, 2, SW), np.float32)
        W = np.asarray(W_ih, np.float32)
        xt[:64, 0, 0:H] = W[:, 0:64].T
        xt[:64, 1, 0:H] = W[:, 64:128].T
        xt[64, 0, 0:H] = np.asarray(b, np.float32)
        xt[:64, 0, H:H + S] = feat[0:64, :]
        xt[:64, 1, H:H + S] = feat[64:128, :]
        xt[64, 0, H:H + S] = mask_flat
        return xt.reshape(65, 2 * SW).astype(E4NP)

    def pack_whh(W_hh):
        # [128, 4, 512]: [p, k, m] = W_hh[m, k*128+p]
        W = np.asarray(W_hh, np.float32)
        w = np.empty((128, HC, H), np.float32)
        for k in range(HC):
            w[:, k, :] = W[:, k * 128:(k + 1) * 128].T
        return w.reshape(128, HC * H).astype(E4NP)

    ident = np.eye(128, dtype=np.float32)
    ida = np.zeros((128, 2, 128), np.float32)
    ida[:, 0, :] = ident
    idb = np.zeros((128, 2, 128), np.float32)
    idb[:, 1, :] = ident
    ida = ida.reshape(128, 256).astype(E4NP)
    idb = idb.reshape(128, 256).astype(E4NP)

    ones = np.ones((1, 128), np.float32).astype(BFNP)
    # wx packed [128, 4*D_PROJ]: [p, k*D_PROJ + j] = Wx[j, k*128+p]
    WxT = np.asarray(Wx, np.float32).T          # [512, 1024]
    wxT = np.ascontiguousarray(
        WxT.reshape(4, 128, D_PROJ).transpose(1, 0, 2).reshape(128, 4 * D_PROJ)
    ).astype(BFNP)
    bxr = np.asarray(bx, np.float32)[None, :].astype(BFNP)

    in_maps = []
    for core in range(N_CORES):
        c = core % 4
        fwd = core < 4
        feat_f, feat_b, mask_flat, invl = per_stripe[c]
        # xT packed [128, 4*XROWS]: [p, k*XROWS + r] = x[row r, k*128+p]
        xcT = x[core * XROWS:(core + 1) * XROWS, :].T    # [512, 256]
        xTp = np.ascontiguousarray(
            xcT.reshape(4, 128, XROWS).transpose(1, 0, 2).reshape(128, 4 * XROWS)
        ).astype(BFNP)
        in_maps.append({
            "xtok": pack_tokens(feat_f if fwd else feat_b, mask_flat,
                                W_ih_f if fwd else W_ih_b,
                                b_f if fwd else b_b),
            "whh": pack_whh(W_hh_f if fwd else W_hh_b),
            "ida": ida,
            "idb": idb,
            "invl": invl,
            "xT": xTp,
            "wxT": wxT,
            "bxrow": bxr,
            "ones": ones,
        })

    return nt, in_maps, gid


def _assemble(res, gid):
    out = np.empty((N_GROUPS, 2 * D_PROJ), np.float32)
    for core in range(N_CORES):
        out[core * XROWS:(core + 1) * XROWS, :D_PROJ] = \
            res[core]["xp"].astype(np.float32)
    for c in range(4):
        for half, core in ((0, c), (1, c + 4)):
            # pooled2 [128, ch*512 + lane] -> [lane, ch*128 + p]
            p2 = res[core]["pooled2"].astype(np.float32)
            pooled = p2.reshape(128, 4, LANES).transpose(2, 1, 0).reshape(LANES, H)
            out[gid[c], D_PROJ + half * H:D_PROJ + (half + 1) * H] = pooled
    return out


def kernel(**inputs):
    nt, in_maps, gid = _prepare(**inputs)
    nc = _get_program(nt)
    res = run_bass_kernel_spmd(nc, in_maps, list(range(N_CORES))).results
    return _assemble(res, gid)


# revision 69
# speedup vs baseline: 1.0016x; 1.0016x over previous
"""Trainium2 Bass kernel: GSpade node embedding (fp8 DoubleRow scan).

Computation (see reference):
  - bidirectional tanh-RNN (hidden 512/dir) over T=32768 tokens grouped into
    N=2048 contiguous ragged segments (sorted group ids in `masks`)
  - mean-pool hidden states per segment -> pooled [N, 1024]
  - out = [x @ Wx.T + bx | pooled]  -> [N, 2048]

Sharding (8 NeuronCores, SPMD single program):
  cores 0-3 forward RNN, 4-7 backward.  Segments sorted by length desc and
  striped by rank (stripe c = core % 4 owns ranks 4i+c -> 512 lanes/core).
  Lanes end-aligned to the shared schedule L_i = len at rank 4i (zero-prefix
  padding keeps h == 0 until the first real token).  Active lanes form a
  prefix; nt[t] shrinks with t.

Per-core scan, all fp8e4m3 with DoubleRow matmuls (2 contraction slots per
partition, 0.5 cycles/row):
  - input proj + bias: tokens packed [65, 2, S]; slots (p<64, i) carry the
    128 token features, slot (64, 0) carries the real-token mask so the bias
    row of the packed W_ih folds the masked bias into the same matmuls.
    4 DR matmuls per part.
  - recurrence: W_hh.T packed [128, 4, 512]; 8 DR matmuls per part
    (4 j-chunks x 2 k-supers) reading the fp8 h of the previous step.
  - tanh: one ACT instruction per part, PSUM -> fp8 h (double-buffered).
  - mean-pool acc: chunks 0-1 via DR identity matmuls into a 2-bank PSUM
    accumulator; chunks 2-3 via DVE adds into SBUF fp32.
Steps with n >= 128 lanes are split into two lane-halves (A/B): ACT runs
half X's tanh while PE runs the other half's matmuls, hiding the serial
recurrence under the ACT tanh floor.  acc work is emitted two parts late so
PE/DVE never queue behind an in-flight ACT.  The bf16 x-projection (fp8
would exceed tolerance) is injected one matmul per part into PE slack,
using the 2 PSUM banks the accumulator does not occupy.
"""

import ml_dtypes
import numpy as np

import concourse.bacc as bacc
import concourse.mybir as mybir
from concourse.tile import TileContext
from concourse.bass_utils import run_bass_kernel_spmd

FP32 = mybir.dt.float32
BF16 = mybir.dt.bfloat16
F8 = mybir.dt.float8e4
DR = mybir.MatmulPerfMode.DoubleRow
Tanh = mybir.ActivationFunctionType.Tanh
E4NP = ml_dtypes.float8_e4m3
BFNP = ml_dtypes.bfloat16

N_GROUPS = 2048
D_SEQ = 128
H = 512           # hidden per direction
HC = 4            # hidden chunks of 128
D_PROJ = 1024
N_CORES = 8
LANES = 512       # segments per scan core
XROWS = N_GROUPS // N_CORES  # x-projection rows per core
SPLIT_MIN = 128   # lane-split steps with n >= this

_program_cache: dict = {}


def _dma_chunks(nt, targets=(768, 1536, 2048)):
    """Group steps into DMA chunks; early chunks smaller so step 0 starts fast."""
    chunks = []
    t0 = 0
    cols = 0
    for t, n in enumerate(nt):
        target = targets[min(len(chunks), len(targets) - 1)]
        if cols > 0 and cols + n > target:
            chunks.append((t0, t))
            t0, cols = t, 0
        cols += n
    chunks.append((t0, len(nt)))
    return chunks


def _parts_of(nt):
    """Flat list of (t, lane_off, nX) parts; big steps split in half."""
    parts = []
    for t, n in enumerate(nt):
        if n >= SPLIT_MIN:
            na = (n + 1) // 2
            parts.append((t, 0, na))
            parts.append((t, na, n - na))
        else:
            parts.append((t, 0, n))
    return parts


def _build_program(nt):
    nt = list(nt)
    steps = len(nt)
    off = np.concatenate([[0], np.cumsum(nt)]).astype(int)
    S = int(off[-1])
    parts = _parts_of(nt)

    nc = bacc.Bacc("TRN2", target_bir_lowering=False, debug=False,
                   num_devices=N_CORES)

    # token stream [65, 2, 512+S(pad16)]: cols 0:512 of each i-half hold the
    # packed W_ih (bias row folded), so one DMA pair covers weights + step 0.
    # The i-half stride must be 16-aligned (DoubleRow LDWEIGHTS ISA rule).
    SW = H + ((S + 15) // 16) * 16
    xtok_d = nc.dram_tensor("xtok", [65, 2 * SW], F8, kind="ExternalInput")
    whh_d = nc.dram_tensor("whh", [128, HC * H], F8, kind="ExternalInput")
    ida_d = nc.dram_tensor("ida", [128, 2 * 128], F8, kind="ExternalInput")
    idb_d = nc.dram_tensor("idb", [128, 2 * 128], F8, kind="ExternalInput")
    # 1/len per lane, duplicated x2 so a single DVE mul covers a chunk pair
    invl_d = nc.dram_tensor("invl", [128, 2 * LANES], FP32, kind="ExternalInput")
    xT_d = nc.dram_tensor("xT", [128, 4 * XROWS], BF16, kind="ExternalInput")
    wxT_d = nc.dram_tensor("wxT", [128, 4 * D_PROJ], BF16, kind="ExternalInput")
    bxrow_d = nc.dram_tensor("bxrow", [1, D_PROJ], BF16, kind="ExternalInput")
    ones_d = nc.dram_tensor("ones", [1, 128], BF16, kind="ExternalInput")

    xp_d = nc.dram_tensor("xp", [XROWS, D_PROJ], BF16, kind="ExternalOutput")
    # lane-major pooled: [p, lane*4 + c] -> feature c*128+p of lane
    pooled2_d = nc.dram_tensor("pooled2", [128, 4 * LANES], BF16, kind="ExternalOutput")

    with TileContext(nc) as tc:
        with (
            tc.tile_pool(name="sb", bufs=1) as sb,
            tc.tile_pool(name="ps", bufs=2, space="PSUM") as psp,
            tc.tile_pool(name="psx", bufs=1, space="PSUM") as psxp,
            tc.tile_pool(name="xps", bufs=1, space="PSUM") as xpsp,
            tc.tile_pool(name="accps", bufs=1, space="PSUM") as accp,
        ):
            # tanh table warm-up: runs under the DMA shadow so the table set
            # is resident before step 0.
            warm_sb = sb.tile([128, 8], FP32, tag="warm", name="warm")
            nc.vector.memset(warm_sb[:, :], 0.0)
            nc.scalar.activation(warm_sb[:, :], warm_sb[:, :], Tanh)

            # ---- persistent SBUF tiles + loads (step-0-critical first) ----
            chunks = _dma_chunks(nt)
            xtok_sb = sb.tile([65, 2 * SW], F8, tag="xtok", name="xtok")

            xtok3_sb = xtok_sb.rearrange("p (i s) -> p i s", i=2)
            xtok3_d = xtok_d.rearrange("p (i s) -> p i s", i=2)

            def xtok_range(a, b):
                # one 3D DMA moves both i-halves (one HWDGE slot, not two)
                nc.sync.dma_start(out=xtok3_sb[:, :, a:b],
                                  in_=xtok3_d[:, :, a:b])

            def xtok_chunk(ci):
                t0, t1 = chunks[ci]
                a = 0 if ci == 0 else int(off[t0]) + H
                xtok_range(a, int(off[t1]) + H)

            xtok_chunk(0)
            whh_sb = sb.tile([128, HC * H], F8, tag="whh", name="whh")
            nc.sync.dma_start(out=whh_sb[:, :], in_=whh_d[:, :])
            if len(chunks) > 1:
                xtok_chunk(1)
            ida_sb = sb.tile([128, 2 * 128], F8, tag="ida", name="ida")
            idb_sb = sb.tile([128, 2 * 128], F8, tag="idb", name="idb")
            nc.sync.dma_start(out=ida_sb[:, :], in_=ida_d[:, :])
            nc.sync.dma_start(out=idb_sb[:, :], in_=idb_d[:, :])
            invl_sb = sb.tile([128, 2 * LANES], FP32, tag="invl", name="invl")
            nc.sync.dma_start(out=invl_sb[:, :], in_=invl_d[:, :])
            for ci in range(2, len(chunks)):
                xtok_chunk(ci)

            ones_sb = sb.tile([1, 128], BF16, tag="ones", name="ones")
            bx_sb = sb.tile([1, D_PROJ], BF16, tag="bx", name="bx")
            nc.sync.dma_start(out=ones_sb[:, :], in_=ones_d[:, :])
            nc.sync.dma_start(out=bx_sb[:, :], in_=bxrow_d[:, :])
            xT_sb = sb.tile([128, 4 * XROWS], BF16, tag="xT", name="xT")
            wx_sb = sb.tile([128, 4 * D_PROJ], BF16, tag="wx", name="wx")
            nc.sync.dma_start(out=xT_sb[:, :], in_=xT_d[:, :])
            nc.sync.dma_start(out=wx_sb[:, :], in_=wxT_d[:, :])

            # scan state (h triple-buffered: ACT(t) writes buf t%3 while
            # lagged acc still reads buf (t-1)%3 without WAR stalls)
            h_sb = [sb.tile([128, HC * LANES], F8, tag=f"h{p}", name=f"h{p}")
                    for p in range(3)]
            acc2_sb = sb.tile([128, 2 * LANES], FP32, tag="acc2", name="acc2")
            xp_sb = [sb.tile([128, D_PROJ], BF16, tag=f"xp{b}", name=f"xpsb{b}")
                     for b in range(2)]
            po_sb = sb.tile([128, 4 * LANES], BF16, tag="po", name="po")  # [p, c, n]

            whh3 = whh_sb.rearrange("p (k m) -> p k m", k=HC)
            ida3 = ida_sb.rearrange("p (i m) -> p i m", i=2)
            idb3 = idb_sb.rearrange("p (i m) -> p i m", i=2)
            xtok3 = xtok3_sb
            acc2v = acc2_sb.rearrange("p (c n) -> p c n", c=2)
            h3 = [h.rearrange("p (c n) -> p c n", c=HC) for h in h_sb]
            pov = po_sb.rearrange("p (c n) -> p c n", c=4)
            pov_d = pooled2_d.rearrange("p (c n) -> p c n", c=4)
            invl2 = invl_sb.rearrange("p (c n) -> p c n", c=2)

            acc_ps = accp.tile([128, 2 * LANES], FP32, tag="acc", name="acc")
            accv = acc_ps.rearrange("p (c n) -> p c n", c=2)

            # ---- x-projection micro-op queue (injected into PE slack) ----
            # 4 groups of (bias + 4 kc) bf16 matmuls; each group -> one DVE
            # copy to bf16 SBUF; DMA per bc block.
            xproj_ops = []

            def _xproj_group(bc, jh):
                ps = xpsp.tile([128, H], FP32, tag="xps", name="xps")
                ops = [lambda ps=ps, bc=bc, jh=jh: nc.tensor.matmul(
                    ps[:, :], ones_sb[0:1, :], bx_sb[0:1, jh * H:(jh + 1) * H],
                    start=True, stop=False)]
                for kc in range(4):
                    ops.append(lambda ps=ps, bc=bc, jh=jh, kc=kc: nc.tensor.matmul(
                        ps[:, :],
                        xT_sb[:, kc * XROWS + bc * 128:kc * XROWS + (bc + 1) * 128],
                        wx_sb[:, kc * D_PROJ + jh * H:kc * D_PROJ + (jh + 1) * H],
                        start=False, stop=(kc == 3)))
                ops.append(lambda ps=ps, bc=bc, jh=jh: nc.vector.tensor_copy(
                    xp_sb[bc][:, jh * H:(jh + 1) * H], ps[:, :]))
                if jh == 1:
                    ops.append(lambda bc=bc: nc.sync.dma_start(
                        out=xp_d[bc * 128:(bc + 1) * 128, :], in_=xp_sb[bc][:, :]))
                return ops

            for bc in range(2):
                for jh in range(2):
                    xproj_ops.extend(_xproj_group(bc, jh))
            xproj_ops.reverse()  # pop() from the front

            # ---- scan: express-lane schedule ----
            # The XL longest lanes (the only ones alive past step TX) run as
            # their own one-instruction-per-step chain ("X") from step 0,
            # interleaved into the bulk ("Y") ACT stream at >= ring-latency
            # spacing, so the serial tail hides under the bulk's tanh
            # throughput instead of running after it.
            import heapq
            TX = next((t for t, n in enumerate(nt) if n < SPLIT_MIN), steps)
            XL = nt[TX] if TX < steps else 0

            seq = []          # (t, o0, nX, kind)
            est_act = 0.0
            x_last = -1e9
            xi = 0
            X_SPACING = 560.0

            def est_cost(n):
                return 3.33 * n + 235.0

            def push_x():
                nonlocal est_act, x_last, xi
                nXp = min(nt[xi], XL)
                seq.append((xi, 0, nXp, "X"))
                x_last = est_act
                est_act += est_cost(nXp)
                xi += 1

            if XL > 0:
                push_x()
            for t in range(TX if XL > 0 else steps):
                nY = nt[t] - XL
                if nY >= SPLIT_MIN:
                    halves = [(XL, (nY + 1) // 2),
                              (XL + (nY + 1) // 2, nY - (nY + 1) // 2)]
                else:
                    halves = [(XL, nY)] if nY > 0 else []
                for (o0, nXp) in halves:
                    if xi < steps and XL > 0 and est_act - x_last >= X_SPACING:
                        push_x()
                    seq.append((t, o0, nXp, "Y"))
                    est_act += est_cost(nXp)
            while XL > 0 and xi < steps:
                push_x()
                est_act = max(est_act, x_last + X_SPACING)

            nparts = len(seq)
            acc_queue = []  # heap of (emit_at_part_idx, seq#, thunk)
            qseq = [0]

            def qpush(at, thunk):
                heapq.heappush(acc_queue, (at, qseq[0], thunk))
                qseq[0] += 1

            def emit_fin_mul(b0, b1, pair):
                # pooled finalize for retired lanes [b0, b1), chunk pair.
                # The SBUF-side pair of the late (express) ranges runs on the
                # idle GPSIMD so the end-of-kernel DVE cascade shortens.
                src = accv[:, 0:2, b0:b1] if pair == 0 else acc2v[:, 0:2, b0:b1]
                nc.vector.tensor_mul(pov[:, 2 * pair:2 * pair + 2, b0:b1],
                                     src, invl2[:, :, b0:b1])

            def emit_fin_dma(b0, b1):
                nc.sync.dma_start(out=pov_d[:, :, b0:b1], in_=pov[:, :, b0:b1])

            def emit_fin(b0, b1):
                for pair in range(2):
                    emit_fin_mul(b0, b1, pair)
                emit_fin_dma(b0, b1)

            def emit_acc(k):
                t, o0, nX, _kind = seq[k]
                hv = h3[t % 3]
                first = (k == 0)
                last_a = (k == nparts - 1)
                # PE: chunks 0,1 -> PSUM acc via DR identity
                for c in range(2):
                    lhs = ida3 if c % 2 == 0 else idb3
                    nc.tensor.matmul(accv[:, c, o0:o0 + nX],
                                     lhs[:, :, :], hv[:, 0:2, o0:o0 + nX],
                                     start=first, stop=last_a,
                                     perf_mode=DR)
                # DVE: chunks 2,3 -> SBUF fp32
                if t == 0:
                    nc.vector.tensor_copy(acc2v[:, :, o0:o0 + nX],
                                          hv[:, 2:4, o0:o0 + nX])
                else:
                    nc.vector.tensor_add(acc2v[:, :, o0:o0 + nX],
                                         acc2v[:, :, o0:o0 + nX],
                                         hv[:, 2:4, o0:o0 + nX])

            FIN_BOUNDS = tuple(sorted({384, 256, 160, XL, 2}, reverse=True))
            fin_y = [LANES]   # descends to XL via Y retirements
            fin_x = [XL]      # descends to 0 via X retirements
            est_pe = [0.0]
            est_a2 = [0.0]
            last_xp = [-2]

            def emit_fins(nxt, k, bound_state, blo, bhi):
                for b in FIN_BOUNDS:
                    if blo <= b < bhi and nxt <= b < bound_state[0]:
                        lo, hi = b, bound_state[0]
                        for pair in range(2):
                            qpush(k + 3 + pair,
                                  lambda lo=lo, hi=hi, pair=pair:
                                  emit_fin_mul(lo, hi, pair))
                        qpush(k + 5, lambda lo=lo, hi=hi: emit_fin_dma(lo, hi))
                        bound_state[0] = b

            for k, (t, o0, nX, kind) in enumerate(seq):
                a = int(off[t])
                hw3 = h3[t % 3]
                hr3 = h3[(t + 2) % 3]
                if kind == "Y":
                    ps = psp.tile([128, HC * 256], FP32, tag="ps", name="ps")
                    ps3 = ps.rearrange("p (c n) -> p c n", c=HC)
                else:
                    ps = psxp.tile([128, HC * max(XL, 1)], FP32, tag="psx",
                                   name="psx")
                    ps3 = ps.rearrange("p (c n) -> p c n", c=HC)

                # psum-group flags follow zero regions (banks): Y tiles span 2
                # banks (chunks 0,1 | 2,3); the X tile fits in one bank.
                starts = (0, 2) if kind == "Y" else (0,)
                stops = (1, 3) if kind == "Y" else (3,)

                # input proj + bias (DR; lhsT is the W_ih block leading the
                # token stream)
                xv = xtok3[:, :, H + a + o0:H + a + o0 + nX]
                for jc in range(HC):
                    nc.tensor.matmul(ps3[:, jc, 0:nX],
                                     xtok3[:, :, jc * 128:(jc + 1) * 128], xv,
                                     start=(jc in starts),
                                     stop=(t == 0 and jc in stops),
                                     perf_mode=DR)
                # recurrence (DR)
                if t > 0:
                    for s in range(2):
                        hk = hr3[:, 2 * s:2 * s + 2, o0:o0 + nX]
                        for jc in range(HC):
                            nc.tensor.matmul(
                                ps3[:, jc, 0:nX],
                                whh3[:, 2 * s:2 * s + 2, jc * 128:(jc + 1) * 128],
                                hk,
                                start=False,
                                stop=(s == 1 and jc in stops),
                                perf_mode=DR)

                # lagged acc (2 parts back) keeps PE/DVE off in-flight ACT.
                # X parts stay lean (proj/rec/tanh only) so the express
                # chain's recurrence never queues behind bulk work on PE.
                est_pe[0] += 2.92 * nX
                if kind == "Y" or k >= nparts - 2:
                    while acc_queue and acc_queue[0][0] <= k:
                        heapq.heappop(acc_queue)[2]()
                    if (xproj_ops and est_a2[0] - est_pe[0] > 450.0
                            and k - last_xp[0] >= 2):
                        xproj_ops.pop()()
                        est_pe[0] += 213.0
                        last_xp[0] = k

                # tanh -> fp8 h
                nc.scalar.activation(hw3[:, 0:HC, o0:o0 + nX],
                                     ps3[:, 0:HC, 0:nX], Tanh)
                est_a2[0] += est_cost(nX)
                qpush(k + 2, lambda k=k: emit_acc(k))

                # progressive pooled finalize once a retirement boundary's acc
                # is emitted (lag 3); split muls keep DVE load smooth.
                if kind == "Y" and o0 + nX == nt[t]:
                    nxt = nt[t + 1] if t + 1 < steps else 0
                    emit_fins(max(nxt, XL), k, fin_y, XL, LANES)
                elif kind == "X":
                    nxt = min(nt[t + 1], XL) if t + 1 < steps else 0
                    emit_fins(nxt, k, fin_x, 0, XL)

            while acc_queue:
                heapq.heappop(acc_queue)[2]()
            while xproj_ops:
                xproj_ops.pop()()
            if fin_y[0] > XL:
                emit_fin(XL, fin_y[0])
            if fin_x[0] > 0:
                emit_fin(0, fin_x[0])

    nc.compile()
    return nc


def _get_program(nt):
    key = tuple(nt)
    if key not in _program_cache:
        _program_cache[key] = _build_program(nt)
    return _program_cache[key]


def _prepare(x, seqs, masks, W_ih_f, W_hh_f, b_f, W_ih_b, W_hh_b, b_b, Wx, bx):
    x = np.asarray(x, np.float32)
    seqs = np.asarray(seqs, np.float32)
    masks = np.asarray(masks).astype(np.int64)

    # ---- segment geometry (host) ----
    lens = np.bincount(masks, minlength=N_GROUPS).astype(np.int64)
    starts_all = np.concatenate([[0], np.cumsum(lens)[:-1]])
    order = np.argsort(-lens, kind="stable")
    sl = lens[order]
    L = sl[0::4].astype(np.int64)        # shared lane schedule (512)
    steps = int(L[0])
    nt = [int((L > t).sum()) for t in range(steps)]
    off = np.concatenate([[0], np.cumsum(nt)]).astype(int)
    S = int(off[-1])

    t_grid = np.arange(steps)[:, None]
    active = t_grid < L[None, :]         # [steps, LANES]

    seqs_pad = np.vstack([np.zeros((1, D_SEQ), np.float32), seqs])

    gid = [order[c::4] for c in range(4)]
    per_stripe = {}
    for c in range(4):
        lens_c = lens[gid[c]]
        starts_c = starts_all[gid[c]]
        pre = (L - lens_c)[None, :]      # zero-prefix length
        real = active & (t_grid >= pre)
        pos = t_grid - pre
        idx_f = np.where(real, starts_c[None, :] + pos, -1)
        idx_b = np.where(real, starts_c[None, :] + lens_c[None, :] - 1 - pos, -1)
        mask_flat = real[active].astype(np.float32)          # [S]
        feat_f = seqs_pad[idx_f[active] + 1].T               # [128, S]
        feat_b = seqs_pad[idx_b[active] + 1].T
        invl = np.ascontiguousarray(
            np.broadcast_to(
                np.tile((1.0 / lens_c).astype(np.float32), 2)[None, :],
                (128, 2 * LANES)))
        per_stripe[c] = (feat_f, feat_b, mask_flat, invl)

    SW = H + ((S + 15) // 16) * 16   # 16-aligned i-half stride (ISA rule)

    def pack_tokens(feat, mask_flat, W_ih, b):
        # [65, 2, SW]: leading 512 cols of each i-half = packed W_ih
        # ([p, i, m] = W_ih[m, 64i+p]; [64, 0, m] = b[m]); then the tokens
        # (slots (p<64, i) = feature 64i+p; (64,0) = mask; (64,1) = 0).
        xt = np.zeros((65, 2, SW), np.float32)
        W = np.asarray(W_ih, np.float32)
        xt[:64, 0, 0:H] = W[:, 0:64].T
        xt[:64, 1, 0:H] = W[:, 64:128].T
        xt[64, 0, 0:H] = np.asarray(b, np.float32)
        xt[:64, 0, H:H + S] = feat[0:64, :]
        xt[:64, 1, H:H + S] = feat[64:128, :]
        xt[64, 0, H:H + S] = mask_flat
        return xt.reshape(65, 2 * SW).astype(E4NP)

    def pack_whh(W_hh):
        # [128, 4, 512]: [p, k, m] = W_hh[m, k*128+p]
        W = np.asarray(W_hh, np.float32)
        w = np.empty((128, HC, H), np.float32)
        for k in range(HC):
            w[:, k, :] = W[:, k * 128:(k + 1) * 128].T
        return w.reshape(128, HC * H).astype(E4NP)

    ident = np.eye(128, dtype=np.float32)
    ida = np.zeros((128, 2, 128), np.float32)
    ida[:, 0, :] = ident
    idb = np.zeros((128, 2, 128), np.float32)
    idb[:, 1, :] = ident
    ida = ida.reshape(128, 256).astype(E4NP)
    idb = idb.reshape(128, 256).astype(E4NP)

    ones = np.ones((1, 128), np.float32).astype(BFNP)
    # wx packed [128, 4*D_PROJ]: [p, k*D_PROJ + j] = Wx[j, k*128+p]
    WxT = np.asarray(Wx, np.float32).T          # [512, 1024]
    wxT = np.ascontiguousarray(
        WxT.reshape(4, 128, D_PROJ).transpose(1, 0, 2).reshape(128, 4 * D_PROJ)
    ).astype(BFNP)
    bxr = np.asarray(bx, np.float32)[None, :].astype(BFNP)

    in_maps = []
    for core in range(N_CORES):
        c = core % 4
        fwd = core < 4
        feat_f, feat_b, mask_flat, invl = per_stripe[c]
        # xT packed [128, 4*XROWS]: [p, k*XROWS + r] = x[row r, k*128+p]
        xcT = x[core * XROWS:(core + 1) * XROWS, :].T    # [512, 256]
        xTp = np.ascontiguousarray(
            xcT.reshape(4, 128, XROWS).transpose(1, 0, 2).reshape(128, 4 * XROWS)
        ).astype(BFNP)
        in_maps.append({
            "xtok": pack_tokens(feat_f if fwd else feat_b, mask_flat,
                                W_ih_f if fwd else W_ih_b,
                                b_f if fwd else b_b),
            "whh": pack_whh(W_hh_f if fwd else W_hh_b),
            "ida": ida,
            "idb": idb,
            "invl": invl,
            "xT": xTp,
            "wxT": wxT,
            "bxrow": bxr,
            "ones": ones,
        })

    return nt, in_maps, gid


def _assemble(res, gid):
    out = np.empty((N_GROUPS, 2 * D_PROJ), np.float32)
    for core in range(N_CORES):
        out[core * XROWS:(core + 1) * XROWS, :D_PROJ] = \
            res[core]["xp"].astype(np.float32)
    for c in range(4):
        for half, core in ((0, c), (1, c + 4)):
            # pooled2 [128, ch*512 + lane] -> [lane, ch*128 + p]
            p2 = res[core]["pooled2"].astype(np.float32)
            pooled = p2.reshape(128, 4, LANES).transpose(2, 1, 0).reshape(LANES, H)
            out[gid[c], D_PROJ + half * H:D_PROJ + (half + 1) * H] = pooled
    return out


def kernel(**inputs):
    nt, in_maps, gid = _prepare(**inputs)
    nc = _get_program(nt)
    res = run_bass_kernel_spmd(nc, in_maps, list(range(N_CORES))).results
    return _assemble(res, gid)


# revision 70
# speedup vs baseline: 1.0022x; 1.0006x over previous
"""Trainium2 Bass kernel: GSpade node embedding (fp8 DoubleRow scan).

Computation (see reference):
  - bidirectional tanh-RNN (hidden 512/dir) over T=32768 tokens grouped into
    N=2048 contiguous ragged segments (sorted group ids in `masks`)
  - mean-pool hidden states per segment -> pooled [N, 1024]
  - out = [x @ Wx.T + bx | pooled]  -> [N, 2048]

Sharding (8 NeuronCores, SPMD single program):
  cores 0-3 forward RNN, 4-7 backward.  Segments sorted by length desc and
  striped by rank (stripe c = core % 4 owns ranks 4i+c -> 512 lanes/core).
  Lanes end-aligned to the shared schedule L_i = len at rank 4i (zero-prefix
  padding keeps h == 0 until the first real token).  Active lanes form a
  prefix; nt[t] shrinks with t.

Per-core scan, all fp8e4m3 with DoubleRow matmuls (2 contraction slots per
partition, 0.5 cycles/row):
  - input proj + bias: tokens packed [65, 2, S]; slots (p<64, i) carry the
    128 token features, slot (64, 0) carries the real-token mask so the bias
    row of the packed W_ih folds the masked bias into the same matmuls.
    4 DR matmuls per part.
  - recurrence: W_hh.T packed [128, 4, 512]; 8 DR matmuls per part
    (4 j-chunks x 2 k-supers) reading the fp8 h of the previous step.
  - tanh: one ACT instruction per part, PSUM -> fp8 h (double-buffered).
  - mean-pool acc: chunks 0-1 via DR identity matmuls into a 2-bank PSUM
    accumulator; chunks 2-3 via DVE adds into SBUF fp32.
Steps with n >= 128 lanes are split into two lane-halves (A/B): ACT runs
half X's tanh while PE runs the other half's matmuls, hiding the serial
recurrence under the ACT tanh floor.  acc work is emitted two parts late so
PE/DVE never queue behind an in-flight ACT.  The bf16 x-projection (fp8
would exceed tolerance) is injected one matmul per part into PE slack,
using the 2 PSUM banks the accumulator does not occupy.
"""

import ml_dtypes
import numpy as np

import concourse.bacc as bacc
import concourse.mybir as mybir
from concourse.tile import TileContext
from concourse.bass_utils import run_bass_kernel_spmd

FP32 = mybir.dt.float32
BF16 = mybir.dt.bfloat16
F8 = mybir.dt.float8e4
DR = mybir.MatmulPerfMode.DoubleRow
Tanh = mybir.ActivationFunctionType.Tanh
E4NP = ml_dtypes.float8_e4m3
BFNP = ml_dtypes.bfloat16

N_GROUPS = 2048
D_SEQ = 128
H = 512           # hidden per direction
HC = 4            # hidden chunks of 128
D_PROJ = 1024
N_CORES = 8
LANES = 512       # segments per scan core
XROWS = N_GROUPS // N_CORES  # x-projection rows per core
SPLIT_MIN = 128   # lane-split steps with n >= this

_program_cache: dict = {}


def _dma_chunks(nt, targets=(768, 1536, 2048)):
    """Group steps into DMA chunks; early chunks smaller so step 0 starts fast."""
    chunks = []
    t0 = 0
    cols = 0
    for t, n in enumerate(nt):
        target = targets[min(len(chunks), len(targets) - 1)]
        if cols > 0 and cols + n > target:
            chunks.append((t0, t))
            t0, cols = t, 0
        cols += n
    chunks.append((t0, len(nt)))
    return chunks


def _parts_of(nt):
    """Flat list of (t, lane_off, nX) parts; big steps split in half."""
    parts = []
    for t, n in enumerate(nt):
        if n >= SPLIT_MIN:
            na = (n + 1) // 2
            parts.append((t, 0, na))
            parts.append((t, na, n - na))
        else:
            parts.append((t, 0, n))
    return parts


def _build_program(nt):
    nt = list(nt)
    steps = len(nt)
    off = np.concatenate([[0], np.cumsum(nt)]).astype(int)
    S = int(off[-1])
    parts = _parts_of(nt)

    nc = bacc.Bacc("TRN2", target_bir_lowering=False, debug=False,
                   num_devices=N_CORES)

    # token stream [65, 2, 512+S(pad16)]: cols 0:512 of each i-half hold the
    # packed W_ih (bias row folded), so one DMA pair covers weights + step 0.
    # The i-half stride must be 16-aligned (DoubleRow LDWEIGHTS ISA rule).
    SW = H + ((S + 15) // 16) * 16
    xtok_d = nc.dram_tensor("xtok", [65, 2 * SW], F8, kind="ExternalInput")
    whh_d = nc.dram_tensor("whh", [128, HC * H], F8, kind="ExternalInput")
    ida_d = nc.dram_tensor("ida", [128, 2 * 128], F8, kind="ExternalInput")
    idb_d = nc.dram_tensor("idb", [128, 2 * 128], F8, kind="ExternalInput")
    # 1/len per lane, duplicated x2 so a single DVE mul covers a chunk pair
    invl_d = nc.dram_tensor("invl", [128, 2 * LANES], FP32, kind="ExternalInput")
    xT_d = nc.dram_tensor("xT", [128, 4 * XROWS], BF16, kind="ExternalInput")
    wxT_d = nc.dram_tensor("wxT", [128, 4 * D_PROJ], BF16, kind="ExternalInput")
    bxrow_d = nc.dram_tensor("bxrow", [1, D_PROJ], BF16, kind="ExternalInput")
    ones_d = nc.dram_tensor("ones", [1, 128], BF16, kind="ExternalInput")

    xp_d = nc.dram_tensor("xp", [XROWS, D_PROJ], BF16, kind="ExternalOutput")
    # lane-major pooled: [p, lane*4 + c] -> feature c*128+p of lane
    pooled2_d = nc.dram_tensor("pooled2", [128, 4 * LANES], BF16, kind="ExternalOutput")

    with TileContext(nc) as tc:
        with (
            tc.tile_pool(name="sb", bufs=1) as sb,
            tc.tile_pool(name="ps", bufs=2, space="PSUM") as psp,
            tc.tile_pool(name="psx", bufs=1, space="PSUM") as psxp,
            tc.tile_pool(name="xps", bufs=1, space="PSUM") as xpsp,
            tc.tile_pool(name="accps", bufs=1, space="PSUM") as accp,
        ):
            # tanh table warm-up: runs under the DMA shadow so the table set
            # is resident before step 0.
            warm_sb = sb.tile([128, 8], FP32, tag="warm", name="warm")
            nc.vector.memset(warm_sb[:, :], 0.0)
            nc.scalar.activation(warm_sb[:, :], warm_sb[:, :], Tanh)

            # ---- persistent SBUF tiles + loads (step-0-critical first) ----
            chunks = _dma_chunks(nt)
            xtok_sb = sb.tile([65, 2 * SW], F8, tag="xtok", name="xtok")

            xtok3_sb = xtok_sb.rearrange("p (i s) -> p i s", i=2)
            xtok3_d = xtok_d.rearrange("p (i s) -> p i s", i=2)

            def xtok_range(a, b):
                # one 3D DMA moves both i-halves (one HWDGE slot, not two)
                nc.sync.dma_start(out=xtok3_sb[:, :, a:b],
                                  in_=xtok3_d[:, :, a:b])

            def xtok_chunk(ci):
                t0, t1 = chunks[ci]
                a = 0 if ci == 0 else int(off[t0]) + H
                xtok_range(a, int(off[t1]) + H)

            xtok_chunk(0)
            whh_sb = sb.tile([128, HC * H], F8, tag="whh", name="whh")
            nc.sync.dma_start(out=whh_sb[:, :], in_=whh_d[:, :])
            if len(chunks) > 1:
                xtok_chunk(1)
            ida_sb = sb.tile([128, 2 * 128], F8, tag="ida", name="ida")
            idb_sb = sb.tile([128, 2 * 128], F8, tag="idb", name="idb")
            nc.sync.dma_start(out=ida_sb[:, :], in_=ida_d[:, :])
            nc.sync.dma_start(out=idb_sb[:, :], in_=idb_d[:, :])
            invl_sb = sb.tile([128, 2 * LANES], FP32, tag="invl", name="invl")
            nc.sync.dma_start(out=invl_sb[:, :], in_=invl_d[:, :])
            for ci in range(2, len(chunks)):
                xtok_chunk(ci)

            ones_sb = sb.tile([1, 128], BF16, tag="ones", name="ones")
            bx_sb = sb.tile([1, D_PROJ], BF16, tag="bx", name="bx")
            nc.sync.dma_start(out=ones_sb[:, :], in_=ones_d[:, :])
            nc.sync.dma_start(out=bx_sb[:, :], in_=bxrow_d[:, :])
            xT_sb = sb.tile([128, 4 * XROWS], BF16, tag="xT", name="xT")
            wx_sb = sb.tile([128, 4 * D_PROJ], BF16, tag="wx", name="wx")
            nc.sync.dma_start(out=xT_sb[:, :], in_=xT_d[:, :])
            nc.sync.dma_start(out=wx_sb[:, :], in_=wxT_d[:, :])

            # scan state (h triple-buffered: ACT(t) writes buf t%3 while
            # lagged acc still reads buf (t-1)%3 without WAR stalls)
            h_sb = [sb.tile([128, HC * LANES], F8, tag=f"h{p}", name=f"h{p}")
                    for p in range(3)]
            acc2_sb = sb.tile([128, 2 * LANES], FP32, tag="acc2", name="acc2")
            xp_sb = [sb.tile([128, D_PROJ], BF16, tag=f"xp{b}", name=f"xpsb{b}")
                     for b in range(2)]
            po_sb = sb.tile([128, 4 * LANES], BF16, tag="po", name="po")  # [p, c, n]

            whh3 = whh_sb.rearrange("p (k m) -> p k m", k=HC)
            ida3 = ida_sb.rearrange("p (i m) -> p i m", i=2)
            idb3 = idb_sb.rearrange("p (i m) -> p i m", i=2)
            xtok3 = xtok3_sb
            acc2v = acc2_sb.rearrange("p (c n) -> p c n", c=2)
            h3 = [h.rearrange("p (c n) -> p c n", c=HC) for h in h_sb]
            pov = po_sb.rearrange("p (c n) -> p c n", c=4)
            pov_d = pooled2_d.rearrange("p (c n) -> p c n", c=4)
            invl2 = invl_sb.rearrange("p (c n) -> p c n", c=2)

            acc_ps = accp.tile([128, 2 * LANES], FP32, tag="acc", name="acc")
            accv = acc_ps.rearrange("p (c n) -> p c n", c=2)

            # ---- x-projection micro-op queue (injected into PE slack) ----
            # 4 groups of (bias + 4 kc) bf16 matmuls; each group -> one DVE
            # copy to bf16 SBUF; DMA per bc block.
            xproj_ops = []

            def _xproj_group(bc, jh):
                ps = xpsp.tile([128, H], FP32, tag="xps", name="xps")
                ops = [lambda ps=ps, bc=bc, jh=jh: nc.tensor.matmul(
                    ps[:, :], ones_sb[0:1, :], bx_sb[0:1, jh * H:(jh + 1) * H],
                    start=True, stop=False)]
                for kc in range(4):
                    ops.append(lambda ps=ps, bc=bc, jh=jh, kc=kc: nc.tensor.matmul(
                        ps[:, :],
                        xT_sb[:, kc * XROWS + bc * 128:kc * XROWS + (bc + 1) * 128],
                        wx_sb[:, kc * D_PROJ + jh * H:kc * D_PROJ + (jh + 1) * H],
                        start=False, stop=(kc == 3)))
                ops.append(lambda ps=ps, bc=bc, jh=jh: nc.vector.tensor_copy(
                    xp_sb[bc][:, jh * H:(jh + 1) * H], ps[:, :]))
                if jh == 1:
                    ops.append(lambda bc=bc: nc.sync.dma_start(
                        out=xp_d[bc * 128:(bc + 1) * 128, :], in_=xp_sb[bc][:, :]))
                return ops

            for bc in range(2):
                for jh in range(2):
                    xproj_ops.extend(_xproj_group(bc, jh))
            xproj_ops.reverse()  # pop() from the front

            # ---- scan: express-lane schedule ----
            # The XL longest lanes (the only ones alive past step TX) run as
            # their own one-instruction-per-step chain ("X") from step 0,
            # interleaved into the bulk ("Y") ACT stream at >= ring-latency
            # spacing, so the serial tail hides under the bulk's tanh
            # throughput instead of running after it.
            import heapq
            TX = next((t for t, n in enumerate(nt) if n < SPLIT_MIN), steps)
            XL = nt[TX] if TX < steps else 0

            seq = []          # (t, o0, nX, kind)
            est_act = 0.0
            x_last = -1e9
            xi = 0
            X_SPACING = 560.0

            def est_cost(n):
                return 3.33 * n + 235.0

            def push_x():
                nonlocal est_act, x_last, xi
                nXp = min(nt[xi], XL)
                seq.append((xi, 0, nXp, "X"))
                x_last = est_act
                est_act += est_cost(nXp)
                xi += 1

            if XL > 0:
                push_x()
            for t in range(TX if XL > 0 else steps):
                nY = nt[t] - XL
                if nY >= SPLIT_MIN:
                    halves = [(XL, (nY + 1) // 2),
                              (XL + (nY + 1) // 2, nY - (nY + 1) // 2)]
                else:
                    halves = [(XL, nY)] if nY > 0 else []
                for (o0, nXp) in halves:
                    if xi < steps and XL > 0 and est_act - x_last >= X_SPACING:
                        push_x()
                    seq.append((t, o0, nXp, "Y"))
                    est_act += est_cost(nXp)
            while XL > 0 and xi < steps:
                push_x()
                est_act = max(est_act, x_last + X_SPACING)

            nparts = len(seq)
            acc_queue = []  # heap of (emit_at_part_idx, seq#, thunk)
            qseq = [0]

            def qpush(at, thunk):
                heapq.heappush(acc_queue, (at, qseq[0], thunk))
                qseq[0] += 1

            def emit_fin_mul(b0, b1, pair):
                # pooled finalize for retired lanes [b0, b1), chunk pair.
                # The SBUF-side pair of the late (express) ranges runs on the
                # idle GPSIMD so the end-of-kernel DVE cascade shortens.
                src = accv[:, 0:2, b0:b1] if pair == 0 else acc2v[:, 0:2, b0:b1]
                nc.vector.tensor_mul(pov[:, 2 * pair:2 * pair + 2, b0:b1],
                                     src, invl2[:, :, b0:b1])

            def emit_fin_dma(b0, b1):
                nc.sync.dma_start(out=pov_d[:, :, b0:b1], in_=pov[:, :, b0:b1])

            def emit_fin(b0, b1):
                for pair in range(2):
                    emit_fin_mul(b0, b1, pair)
                emit_fin_dma(b0, b1)

            def emit_acc(k):
                t, o0, nX, _kind = seq[k]
                hv = h3[t % 3]
                first = (k == 0)
                last_a = (k == nparts - 1)
                # PE: chunks 0,1 -> PSUM acc via DR identity
                for c in range(2):
                    lhs = ida3 if c % 2 == 0 else idb3
                    nc.tensor.matmul(accv[:, c, o0:o0 + nX],
                                     lhs[:, :, :], hv[:, 0:2, o0:o0 + nX],
                                     start=first, stop=last_a,
                                     perf_mode=DR)
                # DVE: chunks 2,3 -> SBUF fp32
                if t == 0:
                    nc.vector.tensor_copy(acc2v[:, :, o0:o0 + nX],
                                          hv[:, 2:4, o0:o0 + nX])
                else:
                    nc.vector.tensor_add(acc2v[:, :, o0:o0 + nX],
                                         acc2v[:, :, o0:o0 + nX],
                                         hv[:, 2:4, o0:o0 + nX])

            FIN_BOUNDS = tuple(sorted({384, 256, 160, XL, 2}, reverse=True))
            fin_y = [LANES]   # descends to XL via Y retirements
            fin_x = [XL]      # descends to 0 via X retirements
            est_pe = [0.0]
            est_a2 = [0.0]
            last_xp = [-2]

            def emit_fins(nxt, k, bound_state, blo, bhi):
                for b in FIN_BOUNDS:
                    if blo <= b < bhi and nxt <= b < bound_state[0]:
                        lo, hi = b, bound_state[0]
                        for pair in range(2):
                            qpush(k + 3 + pair,
                                  lambda lo=lo, hi=hi, pair=pair:
                                  emit_fin_mul(lo, hi, pair))
                        qpush(k + 5, lambda lo=lo, hi=hi: emit_fin_dma(lo, hi))
                        bound_state[0] = b

            for k, (t, o0, nX, kind) in enumerate(seq):
                a = int(off[t])
                hw3 = h3[t % 3]
                hr3 = h3[(t + 2) % 3]
                if kind == "Y":
                    ps = psp.tile([128, HC * 256], FP32, tag="ps", name="ps")
                    ps3 = ps.rearrange("p (c n) -> p c n", c=HC)
                else:
                    ps = psxp.tile([128, HC * max(XL, 1)], FP32, tag="psx",
                                   name="psx")
                    ps3 = ps.rearrange("p (c n) -> p c n", c=HC)

                # psum-group flags follow zero regions (banks): Y tiles span 2
                # banks (chunks 0,1 | 2,3); the X tile fits in one bank.
                starts = (0, 2) if kind == "Y" else (0,)
                stops = (1, 3) if kind == "Y" else (3,)

                # input proj + bias (DR; lhsT is the W_ih block leading the
                # token stream)
                xv = xtok3[:, :, H + a + o0:H + a + o0 + nX]
                for jc in range(HC):
                    nc.tensor.matmul(ps3[:, jc, 0:nX],
                                     xtok3[:, :, jc * 128:(jc + 1) * 128], xv,
                                     start=(jc in starts),
                                     stop=(t == 0 and jc in stops),
                                     perf_mode=DR)
                # recurrence (DR)
                if t > 0:
                    for s in range(2):
                        hk = hr3[:, 2 * s:2 * s + 2, o0:o0 + nX]
                        for jc in range(HC):
                            nc.tensor.matmul(
                                ps3[:, jc, 0:nX],
                                whh3[:, 2 * s:2 * s + 2, jc * 128:(jc + 1) * 128],
                                hk,
                                start=False,
                                stop=(s == 1 and jc in stops),
                                perf_mode=DR)

                # lagged acc (2 parts back) keeps PE/DVE off in-flight ACT.
                # X parts stay lean (proj/rec/tanh only) so the express
                # chain's recurrence never queues behind bulk work on PE.
                est_pe[0] += 2.92 * nX
                if kind == "Y" or k >= nparts - 2:
                    while acc_queue and acc_queue[0][0] <= k:
                        heapq.heappop(acc_queue)[2]()
                    if (xproj_ops and est_a2[0] - est_pe[0] > 450.0
                            and k - last_xp[0] >= 2):
                        xproj_ops.pop()()
                        est_pe[0] += 213.0
                        last_xp[0] = k

                # tanh -> fp8 h
                nc.scalar.activation(hw3[:, 0:HC, o0:o0 + nX],
                                     ps3[:, 0:HC, 0:nX], Tanh)
                est_a2[0] += est_cost(nX)
                qpush(k + 3, lambda k=k: emit_acc(k))

                # progressive pooled finalize once a retirement boundary's acc
                # is emitted (lag 3); split muls keep DVE load smooth.
                if kind == "Y" and o0 + nX == nt[t]:
                    nxt = nt[t + 1] if t + 1 < steps else 0
                    emit_fins(max(nxt, XL), k, fin_y, XL, LANES)
                elif kind == "X":
                    nxt = min(nt[t + 1], XL) if t + 1 < steps else 0
                    emit_fins(nxt, k, fin_x, 0, XL)

            while acc_queue:
                heapq.heappop(acc_queue)[2]()
            while xproj_ops:
                xproj_ops.pop()()
            if fin_y[0] > XL:
                emit_fin(XL, fin_y[0])
            if fin_x[0] > 0:
                emit_fin(0, fin_x[0])

    nc.compile()
    return nc


def _get_program(nt):
    key = tuple(nt)
    if key not in _program_cache:
        _program_cache[key] = _build_program(nt)
    return _program_cache[key]


def _prepare(x, seqs, masks, W_ih_f, W_hh_f, b_f, W_ih_b, W_hh_b, b_b, Wx, bx):
    x = np.asarray(x, np.float32)
    seqs = np.asarray(seqs, np.float32)
    masks = np.asarray(masks).astype(np.int64)

    # ---- segment geometry (host) ----
    lens = np.bincount(masks, minlength=N_GROUPS).astype(np.int64)
    starts_all = np.concatenate([[0], np.cumsum(lens)[:-1]])
    order = np.argsort(-lens, kind="stable")
    sl = lens[order]
    L = sl[0::4].astype(np.int64)        # shared lane schedule (512)
    steps = int(L[0])
    nt = [int((L > t).sum()) for t in range(steps)]
    off = np.concatenate([[0], np.cumsum(nt)]).astype(int)
    S = int(off[-1])

    t_grid = np.arange(steps)[:, None]
    active = t_grid < L[None, :]         # [steps, LANES]

    seqs_pad = np.vstack([np.zeros((1, D_SEQ), np.float32), seqs])

    gid = [order[c::4] for c in range(4)]
    per_stripe = {}
    for c in range(4):
        lens_c = lens[gid[c]]
        starts_c = starts_all[gid[c]]
        pre = (L - lens_c)[None, :]      # zero-prefix length
        real = active & (t_grid >= pre)
        pos = t_grid - pre
        idx_f = np.where(real, starts_c[None, :] + pos, -1)
        idx_b = np.where(real, starts_c[None, :] + lens_c[None, :] - 1 - pos, -1)
        mask_flat = real[active].astype(np.float32)          # [S]
        feat_f = seqs_pad[idx_f[active] + 1].T               # [128, S]
        feat_b = seqs_pad[idx_b[active] + 1].T
        invl = np.ascontiguousarray(
            np.broadcast_to(
                np.tile((1.0 / lens_c).astype(np.float32), 2)[None, :],
                (128, 2 * LANES)))
        per_stripe[c] = (feat_f, feat_b, mask_flat, invl)

    SW = H + ((S + 15) // 16) * 16   # 16-aligned i-half stride (ISA rule)

    def pack_tokens(feat, mask_flat, W_ih, b):
        # [65, 2, SW]: leading 512 cols of each i-half = packed W_ih
        # ([p, i, m] = W_ih[m, 64i+p]; [64, 0, m] = b[m]); then the tokens
        # (slots (p<64, i) = feature 64i+p; (64,0) = mask; (64,1) = 0).
        xt = np.zeros((65, 2, SW), np.float32)
        W = np.asarray(W_ih, np.float32)
        xt[:64, 0, 0:H] = W[:, 0:64].T
        xt[:64, 1, 0:H] = W[:, 64:128].T
        xt[64, 0, 0:H] = np.asarray(b, np.float32)
        xt[:64, 0, H:H + S] = feat[0:64, :]
        xt[:64, 1, H:H + S] = feat[64:128, :]
        xt[64, 0, H:H + S] = mask_flat
        return xt.reshape(65, 2 * SW).astype(E4NP)

    def pack_whh(W_hh):
        # [128, 4, 512]: [p, k, m] = W_hh[m, k*128+p]
        W = np.asarray(W_hh, np.float32)
        w = np.empty((128, HC, H), np.float32)
        for k in range(HC):
            w[:, k, :] = W[:, k * 128:(k + 1) * 128].T
        return w.reshape(128, HC * H).astype(E4NP)

    ident = np.eye(128, dtype=np.float32)
    ida = np.zeros((128, 2, 128), np.float32)
    ida[:, 0, :] = ident
    idb = np.zeros((128, 2, 128), np.float32)
    idb[:, 1, :] = ident
    ida = ida.reshape(128, 256).astype(E4NP)
    idb = idb.reshape(128, 256).astype(E4NP)

    ones = np.ones((1, 128), np.float32).astype(BFNP)
    # wx packed [128, 4*D_PROJ]: [p, k*D_PROJ + j] = Wx[j, k*128+p]
    WxT = np.asarray(Wx, np.float32).T          # [512, 1024]
    wxT = np.ascontiguousarray(
        WxT.reshape(4, 128, D_PROJ).transpose(1, 0, 2).reshape(128, 4 * D_PROJ)
    ).astype(BFNP)
    bxr = np.asarray(bx, np.float32)[None, :].astype(BFNP)

    in_maps = []
    for core in range(N_CORES):
        c = core % 4
        fwd = core < 4
        feat_f, feat_b, mask_flat, invl = per_stripe[c]
        # xT packed [128, 4*XROWS]: [p, k*XROWS + r] = x[row r, k*128+p]
        xcT = x[core * XROWS:(core + 1) * XROWS, :].T    # [512, 256]
        xTp = np.ascontiguousarray(
            xcT.reshape(4, 128, XROWS).transpose(1, 0, 2).reshape(128, 4 * XROWS)
        ).astype(BFNP)
        in_maps.append({
            "xtok": pack_tokens(feat_f if fwd else feat_b, mask_flat,
                                W_ih_f if fwd else W_ih_b,
                                b_f if fwd else b_b),
            "whh": pack_whh(W_hh_f if fwd else W_hh_b),
            "ida": ida,
            "idb": idb,
            "invl": invl,
            "xT": xTp,
            "wxT": wxT,
            "bxrow": bxr,
            "ones": ones,
        })

    return nt, in_maps, gid


def _assemble(res, gid):
    out = np.empty((N_GROUPS, 2 * D_PROJ), np.float32)
    for core in range(N_CORES):
        out[core * XROWS:(core + 1) * XROWS, :D_PROJ] = \
            res[core]["xp"].astype(np.float32)
    for c in range(4):
        for half, core in ((0, c), (1, c + 4)):
            # pooled2 [128, ch*512 + lane] -> [lane, ch*128 + p]
            p2 = res[core]["pooled2"].astype(np.float32)
            pooled = p2.reshape(128, 4, LANES).transpose(2, 1, 0).reshape(LANES, H)
            out[gid[c], D_PROJ + half * H:D_PROJ + (half + 1) * H] = pooled
    return out


def kernel(**inputs):
    nt, in_maps, gid = _prepare(**inputs)
    nc = _get_program(nt)
    res = run_bass_kernel_spmd(nc, in_maps, list(range(N_CORES))).results
    return _assemble(res, gid)
